# revision 19
# baseline (speedup 1.0000x reference)
"""Trainium2 Bass kernel for the LIF-network step (nn_NetworkClass_31018253812098).

Computation (reference, all fp32, N = NN = N_IN = 2048):
    z_out_new = BETA * z_out + z
    v_new     = ALPHA * v + x @ w - V_TH * z + z_out_new @ wrec
    mask      = (v_new[0, :] - V_TH) > 0          # length-2048, from batch row 0
    z_new[i, j] = mask[i]                         # row-broadcast (N == NN)

Strategy: 4x2 grid -- 4 batch shards (512 cols) x 2 feature halves (1024
rows) -- in the TRANSPOSED domain on-chip: per-core tensors are stored
[feature, batch] so the contraction dim of both matmuls lands on SBUF
partitions natively (w / wrec stay natural as the stationary operands,
column-halved per core).  Matmuls run in float32r (TF32, 1 col/cycle vs 4
for float32) which requires an even moving dim, so batch row 0 is prepended
TWICE -- every core computes the full mask column itself (~0.8% extra work,
no collectives).  Because N == NN, feature-tile t of the mask column is
exactly batch-tile t of z_new, so z_new falls out as a per-partition
broadcast, stored as fp8 (values are exactly 0/1).  SPMD uniformity across
the feature halves is achieved purely in DATA: the host permutes the tile
order of z/z_out (own half first) and permutes wrec's row blocks to match,
so one program serves both halves.  All per-core arrays are packed on the
host so every dma_start is ~1 MiB with >=8 KiB contiguous per partition
row (w/wrec are packed chunk-major in exactly the DMA consumption order).
"""

import sys

sys.path.insert(0, "/opt/trn_rl_repo")

import numpy as np

import concourse.mybir as mybir
import concourse.tile as tile
from concourse import bacc, bass_utils

N = 2048
P = 128
NT = N // P          # 16 feature/contraction tiles
NCORES = 8
R, C = 4, 2          # batch shards x feature halves
MS = N // R          # 512-column batch shard
M = MS + 2           # +2 prepended mask columns (fp32r needs an even moving dim)
NH = N // C          # 1024-row feature half
HT = NH // P         # 8 n-tiles per half
MA = 258             # moving piece A (2 mask cols + 256 batch cols)
MB = M - MA          # moving piece B (256)
KC = 4               # k-tiles per weight chunk (1 MiB chunks)
ALPHA = 1.0 - 0.05 / 10.0   # 0.995
BETA = 1.0 - 0.05 / 2.0     # 0.975
V_TH = 2.0

F32 = mybir.dt.float32
F32R = mybir.dt.float32r
F8 = mybir.dt.float8e4


def _build_program():
    # bacc (not raw Bass): its compile pass splits multi-semaphore sync
    # waits that walrus's per-instruction wait limit rejects.
    nc = bacc.Bacc("TRN2", target_bir_lowering=False, debug=False, num_devices=NCORES)

    xt = nc.dram_tensor("xt", [P, NT, M], F32R, kind="ExternalInput").ap()
    vt = nc.dram_tensor("vt", [P, HT, M], F32, kind="ExternalInput").ap()
    zt = nc.dram_tensor("zt", [P, NT, M], F32, kind="ExternalInput").ap()
    zot = nc.dram_tensor("zot", [P, NT, M], F32, kind="ExternalInput").ap()
    # chunk-major: [quarter, kc, p, a, n] in exact DMA consumption order
    wh = nc.dram_tensor("wh", [2, NT // KC, P, KC, MS], F32R, kind="ExternalInput").ap()
    wrech = nc.dram_tensor(
        "wrech", [2, NT // KC, P, KC, MS], F32R, kind="ExternalInput"
    ).ap()

    vout = nc.dram_tensor("vout", [P, HT, MS], F32, kind="ExternalOutput").ap()
    zoout = nc.dram_tensor("zoout", [P, HT, MS], F32, kind="ExternalOutput").ap()
    znout = nc.dram_tensor("znout", [P, HT, MS], F8, kind="ExternalOutput").ap()

    add = mybir.AluOpType.add
    mult = mybir.AluOpType.mult
    is_gt = mybir.AluOpType.is_gt
    Ident = mybir.ActivationFunctionType.Identity

    with tile.TileContext(nc) as tc:
        with (
            tc.tile_pool(name="resident", bufs=1) as res,
            tc.tile_pool(name="zstream", bufs=1) as zs,
            tc.tile_pool(name="whpool", bufs=3) as whp,
            tc.tile_pool(name="wrpool", bufs=3) as wrp,
            tc.tile_pool(name="psum", bufs=8, space="PSUM") as psum_pool,
            tc.tile_pool(name="epi", bufs=2) as epi,
        ):
            xt_s = res.tile([P, NT, M], F32R, tag="xt_s")
            zt_s = res.tile([P, HT, M], F32, tag="zt_s")        # own half only
            vt_s = res.tile([P, HT, M], F32, tag="vt_s")
            zon_r = res.tile([P, NT, M], F32R, tag="zon_r")     # matmul-2 rhs

            nc.sync.dma_start(xt_s[:, 0:4, :], xt[:, 0:4, :])

            psA = psB = None

            def alloc_psums(qq):
                global_ps = (
                    [psum_pool.tile([P, MA], F32, tag="ps", name=f"psA{qq}_{i}") for i in range(4)],
                    [psum_pool.tile([P, MB], F32, tag="ps", name=f"psB{qq}_{i}") for i in range(4)],
                )
                return global_ps

            def mms(wc, rhs_s, kc, start, stop):
                for a in range(KC):
                    k = kc * KC + a
                    for n in range(4):
                        lhsT = wc[:, a, n * P : (n + 1) * P]
                        nc.tensor.matmul(
                            psA[n][:], lhsT=lhsT, rhs=rhs_s[:, k, 0:MA],
                            start=(start and k == 0), stop=(stop and k == NT - 1),
                        )
                        nc.tensor.matmul(
                            psB[n][:], lhsT=lhsT, rhs=rhs_s[:, k, MA:M],
                            start=(start and k == 0), stop=(stop and k == NT - 1),
                        )

            def epilogue(qq):
                for n in range(4):
                    t = qq * 4 + n
                    vo = epi.tile([P, M], F32, tag="vo")
                    nc.vector.scalar_tensor_tensor(
                        vo[:, 0:MA], zt_s[:, t, 0:MA], -V_TH, psA[n][:], mult, add
                    )
                    nc.vector.scalar_tensor_tensor(
                        vo[:, MA:M], zt_s[:, t, MA:M], -V_TH, psB[n][:], mult, add
                    )
                    nc.vector.scalar_tensor_tensor(
                        vo[:], vt_s[:, t, :], ALPHA, vo[:], mult, add
                    )
                    maskv = epi.tile([P, 1], F32, tag="maskv")
                    nc.vector.tensor_scalar(maskv[:], vo[:, 0:1], V_TH, None, is_gt)
                    zn = epi.tile([P, MS], F8, tag="zn")
                    nc.scalar.activation(
                        zn[:], vo[:, 2:M], Ident, bias=maskv[:], scale=0.0
                    )
                    nc.gpsimd.dma_start(vout[:, t, :], vo[:, 2:M])
                    nc.gpsimd.dma_start(znout[:, t, :], zn[:])

            # ---- quarter 0: interleaved wh/wrec chunk consumption ----
            # SP ring: xt, zt, wh | ACT ring: zot, wrec | gpsimd: stores
            psA, psB = alloc_psums(0)
            for kc in range(NT // KC):
                wc = whp.tile([P, KC, MS], F32R, tag="wc", name=f"wc0_{kc}")
                nc.sync.dma_start(wc[:], wh[0, kc])
                if kc == 0:
                    nc.sync.dma_start(zt_s[:, 0:4, :], zt[:, 0:4, :])
                    nc.sync.dma_start(zt_s[:, 4:8, :], zt[:, 4:8, :])
                if kc < 3:
                    nc.sync.dma_start(
                        xt_s[:, 4 * (kc + 1) : 4 * (kc + 2), :],
                        xt[:, 4 * (kc + 1) : 4 * (kc + 2), :],
                    )
                j0 = kc * 4
                zot_q = zs.tile([P, 4, M], F32, tag=f"zot_q{kc % 2}", name=f"zot_q{kc}")
                nc.scalar.dma_start(zot_q[:], zot[:, j0 : j0 + 4, :])
                if kc >= 2:
                    zt_q = zs.tile([P, 4, M], F32, tag=f"zt_q{kc % 2}", name=f"zt_q{kc}")
                    nc.sync.dma_start(zt_q[:], zt[:, j0 : j0 + 4, :])
                for j in range(j0, j0 + 4):
                    ztile = zt_s[:, j, :] if j < HT else zt_q[:, j % 4, :]
                    nc.vector.scalar_tensor_tensor(
                        zon_r[:, j, :], zot_q[:, j % 4, :], BETA, ztile, mult, add
                    )
                    if j < HT:
                        zo_t = zs.tile([P, M], F32, tag="zo_t", bufs=3, name=f"zo_{j}")
                        nc.vector.scalar_tensor_tensor(
                            zo_t[:], zot_q[:, j % 4, :], BETA, ztile, mult, add
                        )
                        nc.gpsimd.dma_start(zoout[:, j, :], zo_t[:, 2:M])
                mms(wc, xt_s, kc, start=True, stop=False)
                rc = wrp.tile([P, KC, MS], F32R, tag="rc", name=f"rc0_{kc}")
                nc.scalar.dma_start(rc[:], wrech[0, kc])
                mms(rc, zon_r, kc, start=False, stop=True)

            nc.sync.dma_start(vt_s[:], vt[:])
            epilogue(0)

            # ---- quarter 1: everything resident except weights ----
            psA, psB = alloc_psums(1)
            for kc in range(NT // KC):
                wc = whp.tile([P, KC, MS], F32R, tag="wc", name=f"wc1_{kc}")
                nc.sync.dma_start(wc[:], wh[1, kc])
                mms(wc, xt_s, kc, start=True, stop=False)
                rc = wrp.tile([P, KC, MS], F32R, tag="rc", name=f"rc1_{kc}")
                nc.scalar.dma_start(rc[:], wrech[1, kc])
                mms(rc, zon_r, kc, start=False, stop=True)
            epilogue(1)

    nc.compile()
    return nc


_PROGRAM_CACHE = {}


def _get_program():
    if "nc" not in _PROGRAM_CACHE:
        _PROGRAM_CACHE["nc"] = _build_program()
    return _PROGRAM_CACHE["nc"]


def _pack(aT, mcols, tile_perm=None):
    """[2048, src-cols] transposed-domain array -> p-major [128, T, M]."""
    a = aT[:, mcols]  # [2048, M]
    t = a.reshape(-1, P, M)  # [T, 128, M]
    if tile_perm is not None:
        t = t[tile_perm]
    return np.ascontiguousarray(t.transpose(1, 0, 2))


def _pack_w(w_h):
    """[2048, 1024] weight half -> chunk-major [2, 4, 128, KC, MS]."""
    # w_h[kc*512 + a*128 + p, q*512 + n] -> wp[q, kc, p, a, n]
    t = w_h.reshape(NT // KC, KC, P, 2, MS)
    return np.ascontiguousarray(t.transpose(3, 0, 2, 1, 4))


def make_in_maps(x, v, z, z_out, w, wrec):
    xT = np.ascontiguousarray(x.T)
    vT = np.ascontiguousarray(v.T)
    zT = np.ascontiguousarray(z.T)
    zoT = np.ascontiguousarray(z_out.T)
    w = np.ascontiguousarray(w, dtype=np.float32)
    wrec = np.ascontiguousarray(wrec, dtype=np.float32)

    wh_packed = [_pack_w(w[:, nh * NH : (nh + 1) * NH]) for nh in range(C)]
    wrech_packed = []
    for nh in range(C):
        perm = np.r_[nh * HT : nh * HT + HT, (1 - nh) * HT : (1 - nh) * HT + HT]
        wr = wrec.reshape(NT, P, N)[perm].reshape(N, N)[:, nh * NH : (nh + 1) * NH]
        wrech_packed.append(_pack_w(wr))

    in_maps = []
    for c in range(NCORES):
        nh, ms = divmod(c, R)
        mcols = np.r_[0, 0, ms * MS : (ms + 1) * MS]
        perm = np.r_[nh * HT : nh * HT + HT, (1 - nh) * HT : (1 - nh) * HT + HT]
        in_maps.append(
            {
                "xt": _pack(xT, mcols),
                "vt": _pack(vT, mcols)[:, nh * HT : nh * HT + HT],
                "zt": _pack(zT, mcols, perm),
                "zot": _pack(zoT, mcols, perm),
                "wh": wh_packed[nh],
                "wrech": wrech_packed[nh],
            }
        )
    return in_maps


def gather(results):
    v_new = np.empty((N, N), np.float32)
    z_new = np.empty((N, N), np.float32)
    z_out_new = np.empty((N, N), np.float32)
    for c, r in enumerate(results):
        nh, ms = divmod(c, R)
        rows = slice(nh * NH, (nh + 1) * NH)
        cols = slice(ms * MS, (ms + 1) * MS)
        vo = r["vout"].transpose(1, 0, 2).reshape(NH, MS)
        zo = r["zoout"].transpose(1, 0, 2).reshape(NH, MS)
        zn = r["znout"].astype(np.float32).transpose(1, 0, 2).reshape(NH, MS)
        v_new[cols, rows] = vo.T  # transposed domain -> natural
        z_out_new[cols, rows] = zo.T
        z_new[rows, cols] = zn  # z_new block is natural already
    return v_new, z_new, z_out_new


def kernel(x, v, z, z_out, w, wrec, _trace=False):
    nc = _get_program()
    in_maps = make_in_maps(x, v, z, z_out, w, wrec)
    res = bass_utils.run_bass_kernel_spmd(
        nc, in_maps, core_ids=list(range(NCORES)), trace=_trace
    )
    out = gather(res.results)
    if _trace:
        return out, res
    return out


# revision 20
# speedup vs baseline: 1.0516x; 1.0516x over previous
"""Trainium2 Bass kernel for the LIF-network step (nn_NetworkClass_31018253812098).

Computation (reference, all fp32, N = NN = N_IN = 2048):
    z_out_new = BETA * z_out + z
    v_new     = ALPHA * v + x @ w - V_TH * z + z_out_new @ wrec
    mask      = (v_new[0, :] - V_TH) > 0          # length-2048, from batch row 0
    z_new[i, j] = mask[i]                         # row-broadcast (N == NN)

Strategy: 4x2 grid -- 4 batch shards (512 cols) x 2 feature halves (1024
rows) -- in the TRANSPOSED domain on-chip: per-core tensors are stored
[feature, batch] so the contraction dim of both matmuls lands on SBUF
partitions natively (w / wrec stay natural as the stationary operands,
column-halved per core).  Matmuls run in float32r (TF32, 1 col/cycle vs 4
for float32) which requires an even moving dim, so batch row 0 is prepended
TWICE -- every core computes the full mask column itself (~0.8% extra work,
no collectives).  Because N == NN, feature-tile t of the mask column is
exactly batch-tile t of z_new, so z_new falls out as a per-partition
broadcast, stored as fp8 (values are exactly 0/1).  SPMD uniformity across
the feature halves is achieved purely in DATA: the host permutes the tile
order of z/z_out (own half first) and permutes wrec's row blocks to match,
so one program serves both halves.  All per-core arrays are packed on the
host so every dma_start is ~1 MiB with >=8 KiB contiguous per partition
row (w/wrec are packed chunk-major in exactly the DMA consumption order).
"""

import sys

sys.path.insert(0, "/opt/trn_rl_repo")

import numpy as np

import concourse.mybir as mybir
import concourse.tile as tile
from concourse import bacc, bass_utils

N = 2048
P = 128
NT = N // P          # 16 feature/contraction tiles
NCORES = 8
R, C = 4, 2          # batch shards x feature halves
MS = N // R          # 512-column batch shard
M = MS + 2           # +2 prepended mask columns (fp32r needs an even moving dim)
NH = N // C          # 1024-row feature half
HT = NH // P         # 8 n-tiles per half
MA = 258             # moving piece A (2 mask cols + 256 batch cols)
MB = M - MA          # moving piece B (256)
KC = 4               # k-tiles per weight chunk (1 MiB chunks)
ALPHA = 1.0 - 0.05 / 10.0   # 0.995
BETA = 1.0 - 0.05 / 2.0     # 0.975
V_TH = 2.0

F32 = mybir.dt.float32
F32R = mybir.dt.float32r
F8 = mybir.dt.float8e4


def _build_program():
    # bacc (not raw Bass): its compile pass splits multi-semaphore sync
    # waits that walrus's per-instruction wait limit rejects.
    nc = bacc.Bacc("TRN2", target_bir_lowering=False, debug=False, num_devices=NCORES)

    xt = nc.dram_tensor("xt", [P, NT, M], F32R, kind="ExternalInput").ap()
    vt = nc.dram_tensor("vt", [P, HT, M], F32, kind="ExternalInput").ap()
    zt = nc.dram_tensor("zt", [P, NT, M], F32, kind="ExternalInput").ap()
    zot = nc.dram_tensor("zot", [P, NT, M], F32, kind="ExternalInput").ap()
    # chunk-major: [quarter, kc, p, a, n] in exact DMA consumption order
    wh = nc.dram_tensor("wh", [2, NT // KC, P, KC, MS], F32R, kind="ExternalInput").ap()
    wrech = nc.dram_tensor(
        "wrech", [2, NT // KC, P, KC, MS], F32R, kind="ExternalInput"
    ).ap()

    vout = nc.dram_tensor("vout", [P, HT, MS], F32, kind="ExternalOutput").ap()
    zoout = nc.dram_tensor("zoout", [P, HT, MS], F32, kind="ExternalOutput").ap()
    znout = nc.dram_tensor("znout", [P, HT, MS], F8, kind="ExternalOutput").ap()

    add = mybir.AluOpType.add
    mult = mybir.AluOpType.mult
    is_gt = mybir.AluOpType.is_gt
    Ident = mybir.ActivationFunctionType.Identity

    with tile.TileContext(nc) as tc:
        with (
            tc.tile_pool(name="resident", bufs=1) as res,
            tc.tile_pool(name="zstream", bufs=2) as zs,
            tc.tile_pool(name="wchunk", bufs=3) as wpool,
            tc.tile_pool(name="wrchunk", bufs=3) as wrpool,
            tc.tile_pool(name="psum", bufs=8, space="PSUM") as psum_pool,
            tc.tile_pool(name="epi", bufs=2) as epi,
        ):
            xt_s = res.tile([P, NT, M], F32R, tag="xt_s")
            zt_s = res.tile([P, HT, M], F32, tag="zt_s")        # own half only
            vt_s = res.tile([P, HT, M], F32, tag="vt_s")
            zon_r = res.tile([P, NT, M], F32R, tag="zon_r")     # matmul-2 rhs

            def mm_block(src, rhs_s, q, first):
                for kc in range(NT // KC):
                    if first:
                        wc = wpool.tile([P, KC, MS], F32R, tag="wc")
                        nc.sync.dma_start(wc[:], src[q, kc])
                    else:
                        wc = wrpool.tile([P, KC, MS], F32R, tag="rc")
                        nc.scalar.dma_start(wc[:], src[q, kc])
                    for a in range(KC):
                        k = kc * KC + a
                        for n in range(4):
                            lhsT = wc[:, a, n * P : (n + 1) * P]
                            nc.tensor.matmul(
                                psA[n][:],
                                lhsT=lhsT,
                                rhs=rhs_s[:, k, 0:MA],
                                start=(first and k == 0),
                                stop=((not first) and k == NT - 1),
                            )
                            nc.tensor.matmul(
                                psB[n][:],
                                lhsT=lhsT,
                                rhs=rhs_s[:, k, MA:M],
                                start=(first and k == 0),
                                stop=((not first) and k == NT - 1),
                            )

            # --- emission order = DMA queue priority: critical path first ---
            # xt quarter 0, then MM1-q0 chunks interleaved with the
            # zon-build inputs (zt own half, zot stream), so the PE starts
            # at ~3 us and zon is ready right after MM1 q0 drains.
            nc.sync.dma_start(xt_s[:, 0:4, :], xt[:, 0:4, :])

            q = 0
            psA = [psum_pool.tile([P, MA], F32, tag="ps", name=f"psA0_{i}") for i in range(4)]
            psB = [psum_pool.tile([P, MB], F32, tag="ps", name=f"psB0_{i}") for i in range(4)]

            # interleave: wc kc / xt quarter / zon inputs
            def zon_quarter(jq):
                """Load zt/zot for tiles jq*4..jq*4+4 and build zon there."""
                for j in range(jq * 4, jq * 4 + 4):
                    if j < HT:
                        ztile = zt_s[:, j, :]
                    else:
                        if j % 4 == 0:
                            zon_quarter.zt_q = zs.tile(
                                [P, 4, M], F32, tag=f"zt_q{jq}", bufs=1, name=f"zt_q{jq}"
                            )
                            nc.sync.dma_start(
                                zon_quarter.zt_q[:], zt[:, j : j + 4, :]
                            )
                        ztile = zon_quarter.zt_q[:, j % 4, :]
                    if j % 4 == 0:
                        zon_quarter.zot_q = zs.tile(
                            [P, 4, M], F32, tag=f"zot_q{jq % 2}", bufs=1, name=f"zot_q{jq}"
                        )
                        nc.sync.dma_start(zon_quarter.zot_q[:], zot[:, j : j + 4, :])
                    zot_t = zon_quarter.zot_q[:, j % 4, :]
                    # rounded at the producer: the PE consumes f32r directly
                    nc.vector.scalar_tensor_tensor(
                        zon_r[:, j, :], zot_t, BETA, ztile, mult, add
                    )
                    if j < HT:
                        # exact f32 for this core's z_out_new output
                        zo_t = zs.tile([P, M], F32, tag="zo_t")
                        nc.vector.scalar_tensor_tensor(
                            zo_t[:], zot_t, BETA, ztile, mult, add
                        )
                        nc.gpsimd.dma_start(zoout[:, j, :], zo_t[:, 2:M])

            # MM1 q0 interleaved with xt quarters + zon quarters
            first = True
            for kc in range(NT // KC):
                wc = wpool.tile([P, KC, MS], F32R, tag="wc")
                nc.sync.dma_start(wc[:], wh[q, kc])
                if kc == 0:
                    # zt own half -- needed by zon build + epilogue
                    nc.sync.dma_start(zt_s[:, 0:4, :], zt[:, 0:4, :])
                    nc.sync.dma_start(zt_s[:, 4:8, :], zt[:, 4:8, :])
                if kc < 3:
                    nc.sync.dma_start(
                        xt_s[:, 4 * (kc + 1) : 4 * (kc + 2), :],
                        xt[:, 4 * (kc + 1) : 4 * (kc + 2), :],
                    )
                zon_quarter(kc)
                for a in range(KC):
                    k = kc * KC + a
                    for n in range(4):
                        lhsT = wc[:, a, n * P : (n + 1) * P]
                        nc.tensor.matmul(
                            psA[n][:], lhsT=lhsT, rhs=xt_s[:, k, 0:MA],
                            start=(k == 0), stop=False,
                        )
                        nc.tensor.matmul(
                            psB[n][:], lhsT=lhsT, rhs=xt_s[:, k, MA:M],
                            start=(k == 0), stop=False,
                        )

            nc.sync.dma_start(vt_s[:], vt[:])
            mm_block(wrech, zon_r, 0, first=False)

            def epilogue(qq):
                for n in range(4):
                    t = qq * 4 + n
                    vo = epi.tile([P, M], F32, tag="vo")
                    nc.vector.scalar_tensor_tensor(
                        vo[:, 0:MA], zt_s[:, t, 0:MA], -V_TH, psA[n][:], mult, add
                    )
                    nc.vector.scalar_tensor_tensor(
                        vo[:, MA:M], zt_s[:, t, MA:M], -V_TH, psB[n][:], mult, add
                    )
                    nc.vector.scalar_tensor_tensor(
                        vo[:], vt_s[:, t, :], ALPHA, vo[:], mult, add
                    )
                    maskv = epi.tile([P, 1], F32, tag="maskv")
                    nc.vector.tensor_scalar(maskv[:], vo[:, 0:1], V_TH, None, is_gt)
                    zn = epi.tile([P, MS], F8, tag="zn")
                    nc.gpsimd.tensor_scalar(
                        zn[:], vo[:, 2:M], 0.0, maskv[:], mult, add
                    )
                    nc.gpsimd.dma_start(vout[:, t, :], vo[:, 2:M])
                    nc.gpsimd.dma_start(znout[:, t, :], zn[:])

            epilogue(0)

            q = 1
            psA = [psum_pool.tile([P, MA], F32, tag="ps", name=f"psA1_{i}") for i in range(4)]
            psB = [psum_pool.tile([P, MB], F32, tag="ps", name=f"psB1_{i}") for i in range(4)]
            mm_block(wh, xt_s, 1, first=True)
            mm_block(wrech, zon_r, 1, first=False)
            epilogue(1)

    nc.compile()
    return nc


_PROGRAM_CACHE = {}


def _get_program():
    if "nc" not in _PROGRAM_CACHE:
        _PROGRAM_CACHE["nc"] = _build_program()
    return _PROGRAM_CACHE["nc"]


def _pack(aT, mcols, tile_perm=None):
    """[2048, src-cols] transposed-domain array -> p-major [128, T, M]."""
    a = aT[:, mcols]  # [2048, M]
    t = a.reshape(-1, P, M)  # [T, 128, M]
    if tile_perm is not None:
        t = t[tile_perm]
    return np.ascontiguousarray(t.transpose(1, 0, 2))


def _pack_w(w_h):
    """[2048, 1024] weight half -> chunk-major [2, 4, 128, KC, MS]."""
    # w_h[kc*512 + a*128 + p, q*512 + n] -> wp[q, kc, p, a, n]
    t = w_h.reshape(NT // KC, KC, P, 2, MS)
    return np.ascontiguousarray(t.transpose(3, 0, 2, 1, 4))


def make_in_maps(x, v, z, z_out, w, wrec):
    xT = np.ascontiguousarray(x.T)
    vT = np.ascontiguousarray(v.T)
    zT = np.ascontiguousarray(z.T)
    zoT = np.ascontiguousarray(z_out.T)
    w = np.ascontiguousarray(w, dtype=np.float32)
    wrec = np.ascontiguousarray(wrec, dtype=np.float32)

    wh_packed = [_pack_w(w[:, nh * NH : (nh + 1) * NH]) for nh in range(C)]
    wrech_packed = []
    for nh in range(C):
        perm = np.r_[nh * HT : nh * HT + HT, (1 - nh) * HT : (1 - nh) * HT + HT]
        wr = wrec.reshape(NT, P, N)[perm].reshape(N, N)[:, nh * NH : (nh + 1) * NH]
        wrech_packed.append(_pack_w(wr))

    in_maps = []
    for c in range(NCORES):
        nh, ms = divmod(c, R)
        mcols = np.r_[0, 0, ms * MS : (ms + 1) * MS]
        perm = np.r_[nh * HT : nh * HT + HT, (1 - nh) * HT : (1 - nh) * HT + HT]
        in_maps.append(
            {
                "xt": _pack(xT, mcols),
                "vt": _pack(vT, mcols)[:, nh * HT : nh * HT + HT],
                "zt": _pack(zT, mcols, perm),
                "zot": _pack(zoT, mcols, perm),
                "wh": wh_packed[nh],
                "wrech": wrech_packed[nh],
            }
        )
    return in_maps


def gather(results):
    v_new = np.empty((N, N), np.float32)
    z_new = np.empty((N, N), np.float32)
    z_out_new = np.empty((N, N), np.float32)
    for c, r in enumerate(results):
        nh, ms = divmod(c, R)
        rows = slice(nh * NH, (nh + 1) * NH)
        cols = slice(ms * MS, (ms + 1) * MS)
        vo = r["vout"].transpose(1, 0, 2).reshape(NH, MS)
        zo = r["zoout"].transpose(1, 0, 2).reshape(NH, MS)
        zn = r["znout"].astype(np.float32).transpose(1, 0, 2).reshape(NH, MS)
        v_new[cols, rows] = vo.T  # transposed domain -> natural
        z_out_new[cols, rows] = zo.T
        z_new[rows, cols] = zn  # z_new block is natural already
    return v_new, z_new, z_out_new


def kernel(x, v, z, z_out, w, wrec, _trace=False):
    nc = _get_program()
    in_maps = make_in_maps(x, v, z, z_out, w, wrec)
    res = bass_utils.run_bass_kernel_spmd(
        nc, in_maps, core_ids=list(range(NCORES)), trace=_trace
    )
    out = gather(res.results)
    if _trace:
        return out, res
    return out
